# revision 26
# baseline (speedup 1.0000x reference)
"""HIoU kernel for Trainium2 (8 NeuronCores, Bass/Tile).

Algorithm (interval method, exact integer pixel counts):
  Each of the 2^8 = 256 piece masks per image is an intersection of 8
  half-planes.  On a fixed pixel row y, each half-plane constraint is
  "x > t" or "x < t" with t = u*y + v (lines normalized so alpha > 0),
  so every piece's row-mask is an integer interval [L, H).  Then
     inter[m, n] = sum_y overlap(interval_m(y), interval_n(y))
     aP[m]       = sum_y len(interval_m(y))
  which replaces the reference's 256 x 256 x 160000 mask GEMM with
  ~256 x 256 x 400 interval-overlap ops.

  Device layout per core (rows sharded, 50 rows/core):
   - t table:        [128 rows, 16 lines]   (stt: u*y + v, clamped)
   - lo/hi trees:    [128 rows, 256 pieces] fp32 via tensor_scalar max/min
   - integerize:     floor/ceil via AluOp.mod, clamp to [0,400], cast fp16
   - gt tables       -> replicated [128, 50, 256] via DRAM-bounce DMA
   - pred tables     -> transposed [piece, row] via PE transpose
   - inner loop (per row y, per piece-half h):
       maxL = ts_max(LOrep[y], scalar=PLO[:,h,y])           (DVE 4x)
       D    = stt(HIrep[y], scalar=PH[:,h,y], min, sub maxL) (DVE 2x)
       D'   = relu(D)                                        (ACT)
       pacc[h] += I^T @ D'                                   (PE, PSUM fp32)
  Partial inter/aP/aG tables are returned per core and reduced on host.
"""

import sys

import numpy as np

if "/opt/trn_rl_repo" not in sys.path:
    sys.path.insert(0, "/opt/trn_rl_repo")

_NCORES = 8
_HGT = 400
_ROWS = _HGT // _NCORES  # 50 rows per core
_RGRP = 10  # rows per replication DMA group
_NGRP = _ROWS // _RGRP

_CACHE = {}


def _build_nc():
    import concourse.bass as bass
    import concourse.mybir as mybir
    import concourse.tile as tile
    import concourse.tile_sem_assignment as tsa
    from concourse.masks import make_identity

    # Fewer SWDGE proc lanes -> the kernel-tail Drain carries fewer sync
    # waits (walrus CTRL_NO encoding rejects >8 waits).
    tsa.NUM_SWDGE_GLOBAL_SEMS = 2

    # walrus rejects >1 sync wait on most encodings in this toolchain; the
    # Tile tail drain waits on every busy proc at once.  Pre-absorb each
    # proc's final tick onto its own single-wait SP nop so the drain's own
    # waits all elide.
    import bass_rust as _br

    def _split_drain_and_barrier(self, tick_clock, wait_clock):
        ncx = self.nc
        gc = tick_clock.global_clock
        nprocs = len(tsa.PROC_NAME_TO_IDX) if hasattr(tsa, "PROC_NAME_TO_IDX") else 27
        ticks = [int(x) for x in repr(gc).split("[")[1].split("]")[0].split(",")]
        for proc in range(min(nprocs, len(ticks))):
            t = ticks[proc]
            if t <= 0:
                continue
            nop = ncx.sync.nop()
            part = _br.VectorClock()
            part.require_at_least(proc, t)
            wait_clock.add_sem_waits(nop.ins, _br.ScopedClock({None: part}))
        # The nops above (same engine, program order) already enforce every
        # proc's final tick; the drain itself needs no waits of its own.
        ncx.sync.drain()
        ncx.all_engine_barrier()
        assert self.sems is not None
        popped = ncx._tile_sem_poison_stack.pop()
        assert popped is self._sem_poison
        ncx.clear_and_free_semaphores(list(self.sems.allocated().values()))
        ncx.all_engine_barrier()

    tile.TileContext._drain_and_barrier = _split_drain_and_barrier

    dt = mybir.dt
    Alu = mybir.AluOpType
    Act = mybir.ActivationFunctionType
    AX = mybir.AxisListType
    f32, f16 = dt.float32, dt.float16

    nc = bass.Bass("TRN2", debug=False, num_devices=_NCORES)

    uvy = nc.dram_tensor("uvy", [128, 33], f32, kind="ExternalInput")
    out_inter = nc.dram_tensor("out_inter", [256, 256], f32, kind="ExternalOutput")
    out_ap = nc.dram_tensor("out_ap", [128, 2], f32, kind="ExternalOutput")
    out_ag = nc.dram_tensor("out_ag", [1, 256], f32, kind="ExternalOutput")

    with tile.TileContext(nc) as tc:
        with (
            tc.tile_pool(name="const", bufs=1) as const,
            tc.tile_pool(name="tabs", bufs=1) as tabs,
            tc.tile_pool(name="scratch", bufs=2) as scratch,
            tc.tile_pool(name="loop", bufs=3) as loop,
            tc.tile_pool(name="psacc", bufs=1, space="PSUM") as psacc,
            tc.tile_pool(name="psx", bufs=2, space="PSUM") as psx,
            tc.tile_pool(name="dram", bufs=1, space="DRAM") as dram,
        ):
            # ---- stage A: t table ---------------------------------------
            uvy_t = const.tile([128, 33], f32)
            nc.gpsimd.dma_start(uvy_t[:], uvy[:])

            t_t = const.tile([128, 16], f32)
            # t = u*y + v
            nc.vector.scalar_tensor_tensor(
                out=t_t[:], in0=uvy_t[:, 0:16], scalar=uvy_t[:, 32:33],
                in1=uvy_t[:, 16:32], op0=Alu.mult, op1=Alu.add,
            )
            # clamp to [-2, 401]
            nc.vector.tensor_scalar(
                out=t_t[:], in0=t_t[:], scalar1=-2.0, scalar2=401.0,
                op0=Alu.max, op1=Alu.min,
            )

            # ---- stage B+C: per-image lo/hi tree + integerize -----------
            def build_tables(img):  # img 0 = pred (lines 0-7), 1 = gt (8-15)
                LO = tabs.tile([128, 256], f32, tag=f"LO{img}")
                HI = tabs.tile([128, 256], f32, tag=f"HI{img}")
                nc.vector.memset(LO[:, 0:1], -2.0)
                nc.vector.memset(HI[:, 0:1], 401.0)
                for k in range(8):
                    ln = 8 * img + k
                    w = 1 << k
                    tcol = t_t[:, ln : ln + 1]
                    # neg half: copy lo, hi = min(parent hi, t)
                    nc.vector.tensor_copy(LO[:, w : 2 * w], LO[:, 0:w])
                    nc.vector.tensor_scalar(
                        out=HI[:, w : 2 * w], in0=HI[:, 0:w],
                        scalar1=tcol, scalar2=None, op0=Alu.min,
                    )
                    # pos half: lo = max(lo, t) in place
                    nc.vector.tensor_scalar(
                        out=LO[:, 0:w], in0=LO[:, 0:w],
                        scalar1=tcol, scalar2=None, op0=Alu.max,
                    )
                # Integerize without AluOp.mod (invalid on TS encoding):
                # RNE-to-integer via +2^23, with a 0.499 downshift so
                # floor(x) = RNE(x - c) (ties land safely; error band ~1e-3
                # of a pixel is negligible for HIoU).
                C = 0.4990234375
                BIG = 8388608.0
                tmp = scratch.tile([128, 256], f32, tag="itmp")
                Lf16 = tabs.tile([128, 256], f16, tag=f"L16_{img}")
                Hf16 = tabs.tile([128, 256], f16, tag=f"H16_{img}")
                # L = clamp(floor(LO) + 1, 0, 400)
                nc.vector.tensor_scalar(
                    out=tmp[:], in0=LO[:], scalar1=-C, scalar2=BIG,
                    op0=Alu.add, op1=Alu.add,
                )
                nc.vector.tensor_scalar(
                    out=tmp[:], in0=tmp[:], scalar1=BIG - 1.0, scalar2=0.0,
                    op0=Alu.subtract, op1=Alu.max,
                )
                nc.vector.tensor_scalar(
                    out=Lf16[:], in0=tmp[:], scalar1=400.0, scalar2=None,
                    op0=Alu.min,
                )
                # H = 400 - clamp(floor(400 - HI), 0, 400)  (= clamp(ceil(HI),0,400))
                nc.vector.tensor_scalar(
                    out=tmp[:], in0=HI[:], scalar1=-1.0, scalar2=400.0 - C,
                    op0=Alu.mult, op1=Alu.add,
                )
                nc.vector.tensor_scalar(
                    out=tmp[:], in0=tmp[:], scalar1=BIG, scalar2=BIG,
                    op0=Alu.add, op1=Alu.subtract,
                )
                nc.vector.tensor_scalar(
                    out=tmp[:], in0=tmp[:], scalar1=0.0, scalar2=400.0,
                    op0=Alu.max, op1=Alu.min,
                )
                nc.vector.tensor_scalar(
                    out=Hf16[:], in0=tmp[:], scalar1=-1.0, scalar2=400.0,
                    op0=Alu.mult, op1=Alu.add,
                )
                return Lf16, Hf16

            # gt first so its replication DMAs start early
            gL, gH = build_tables(1)
            pL, pH = build_tables(0)

            # ---- stage D1: replicate gt tables across partitions --------
            # DRAM bounce then broadcast-read back, in row groups.
            gLd = dram.tile([_ROWS, 256], f16)
            gHd = dram.tile([_ROWS, 256], f16)
            btouch = const.tile([1, 2], f16)
            nc.gpsimd.tensor_copy(btouch[:, 0:1], gL[0:1, 0:1])
            nc.gpsimd.tensor_copy(btouch[:, 1:2], gH[0:1, 0:1])
            nc.gpsimd.dma_start(gLd[:], gL[:_ROWS, :])
            nc.gpsimd.dma_start(gHd[:], gH[:_ROWS, :])
            LOrep = []
            HIrep = []
            for g in range(_NGRP):
                lr = tabs.tile([128, _RGRP, 256], f16, tag=f"LOrep{g}")
                hr = tabs.tile([128, _RGRP, 256], f16, tag=f"HIrep{g}")
                sl = slice(g * _RGRP, (g + 1) * _RGRP)
                nc.gpsimd.dma_start(
                    lr[:], gLd[sl, :].unsqueeze(0).broadcast_to([128, _RGRP, 256])
                )
                nc.gpsimd.dma_start(
                    hr[:], gHd[sl, :].unsqueeze(0).broadcast_to([128, _RGRP, 256])
                )
                # DVE touch: absorb the multi-queue DMA waits on a copy
                # encoding so the inner-loop STT never carries >1 sync wait.
                touch = scratch.tile([128, 2], f16, tag="touch")
                nc.vector.tensor_copy(touch[:, 0:1], lr[:, 0, 0:1])
                nc.vector.tensor_copy(touch[:, 1:2], hr[:, 0, 0:1])
                LOrep.append(lr)
                HIrep.append(hr)

            # ---- stage D2: transpose pred tables ------------------------
            ident = const.tile([128, 128], f16)
            make_identity(nc, ident[:])
            PLO = tabs.tile([128, 2, 128], f32)
            PH = tabs.tile([128, 2, 128], f32)
            for h in range(2):
                for tab, dst in ((pL, PLO), (pH, PH)):
                    pst = psx.tile([128, 128], f16, tag="tpose")
                    nc.tensor.transpose(
                        out=pst[:], in_=tab[:, h * 128 : (h + 1) * 128],
                        identity=ident[:],
                    )
                    nc.vector.tensor_copy(dst[:, h, :], pst[:])

            # ---- stage E: inner loop ------------------------------------
            pacc = [psacc.tile([128, 256], f32, name=f"acc{h}", tag=f"acc{h}") for h in range(2)]
            # All-DVE chain, one fresh staging tile per iteration: every
            # instruction carries at most one sync wait (walrus limit on
            # TensorScalarPtr encodings).  pacc accumulates -relu(D); the
            # sign is fixed on copy-out.
            for y in range(_ROWS):
                g, yy = divmod(y, _RGRP)
                for h in range(2):
                    T = tabs.tile([128, 256], f16, name=f"T{y}_{h}", tag=f"T{y}_{h}")
                    # T = min(gtHI, predHI)
                    nc.vector.tensor_scalar(
                        out=T[:], in0=HIrep[g][:, yy, :],
                        scalar1=PH[:, h, y : y + 1], scalar2=None, op0=Alu.min,
                    )
                    # T = max(gtLO, predLO) - T   (= -D)
                    nc.vector.scalar_tensor_tensor(
                        out=T[:], in0=LOrep[g][:, yy, :],
                        scalar=PLO[:, h, y : y + 1], in1=T[:],
                        op0=Alu.max, op1=Alu.subtract,
                    )
                    # T = min(T, 0) = -relu(D)
                    nc.vector.tensor_scalar(
                        out=T[:], in0=T[:], scalar1=0.0, scalar2=None, op0=Alu.min,
                    )
                    nc.tensor.matmul(
                        pacc[h][:], ident[:], T[:],
                        start=(y == 0), stop=(y == _ROWS - 1),
                    )

            # ---- stage F: aP / aG partials ------------------------------
            # aP[m,h] = sum_y relu(PH - PLO)  over this core's rows
            apd = scratch.tile([128, 2, _ROWS], f16, tag="apd")
            ap_out = tabs.tile([128, 2], f32)
            for h in range(2):
                nc.vector.tensor_tensor(
                    out=apd[:, h, :], in0=PH[:, h, :_ROWS], in1=PLO[:, h, :_ROWS],
                    op=Alu.subtract,
                )
                nc.vector.tensor_scalar(
                    out=apd[:, h, :], in0=apd[:, h, :], scalar1=0.0, scalar2=0.0,
                    op0=Alu.max, op1=Alu.add, accum_out=ap_out[:, h : h + 1],
                )
            # aG[n] = sum_rows relu(gH - gL) via ones-masked matmul
            ones_mask = const.tile([128, 1], f16)
            nc.vector.memset(ones_mask[:], 0.0)
            nc.vector.memset(ones_mask[:_ROWS, :], 1.0)
            agd = scratch.tile([128, 256], f16, tag="agd")
            nc.vector.tensor_tensor(out=agd[:], in0=gH[:], in1=gL[:], op=Alu.subtract)
            nc.scalar.activation(out=agd[:], in_=agd[:], func=Act.Relu)
            pag = psx.tile([1, 256], f32, tag="pag")
            nc.tensor.matmul(pag[:], ones_mask[:], agd[:], start=True, stop=True)
            ag_out = tabs.tile([1, 256], f32)
            nc.scalar.activation(out=ag_out[:], in_=pag[:], func=Act.Copy)

            # ---- stage G: write outputs ---------------------------------
            # gpsimd touch ops absorb cross-engine waits so each DMA trigger
            # carries at most one sync wait (walrus pseudo-DMA limit).
            gtouch = const.tile([1, 8], f32)
            acc_sbs = []
            for h in range(2):
                acc_sb = scratch.tile([128, 256], f32, tag=f"acc_sb{h}", name=f"acc_sb{h}")
                nc.scalar.mul(acc_sb[:], pacc[h][:], -1.0)
                acc_sbs.append(acc_sb)
            nc.gpsimd.tensor_copy(gtouch[:, 0:1], acc_sbs[0][0:1, 0:1])
            nc.gpsimd.tensor_copy(gtouch[:, 1:2], acc_sbs[1][0:1, 0:1])
            nc.gpsimd.tensor_copy(gtouch[:, 2:3], ap_out[0:1, 0:1])
            nc.gpsimd.tensor_copy(gtouch[:, 4:5], ap_out[0:1, 1:2])
            nc.gpsimd.tensor_copy(gtouch[:, 3:4], ag_out[0:1, 0:1])
            for h in range(2):
                nc.gpsimd.dma_start(out_inter[h * 128 : (h + 1) * 128, :], acc_sbs[h][:])
            nc.gpsimd.dma_start(out_ap[:], ap_out[:])
            nc.gpsimd.dma_start(out_ag[:], ag_out[:])

    return nc


def _host_params(pred_pts, gt_pts):
    """Per-line (u, v) with t(y) = u*y + v; alpha normalized positive."""
    pts = np.concatenate(
        [np.asarray(pred_pts, np.float32), np.asarray(gt_pts, np.float32)], axis=0
    )
    x1, y1, x2, y2 = pts[:, 0], pts[:, 1], pts[:, 2], pts[:, 3]
    vert = x1 == x2
    a = (y1 - y2) / np.where(vert, np.float32(1.0), x1 - x2)
    c = -a * x1 + y1
    # dist sign ~ alpha*x + beta*y + gamma
    alpha = np.where(vert, np.float32(1.0), a).astype(np.float64)
    beta = np.where(vert, np.float32(0.0), np.float32(-1.0)).astype(np.float64)
    gamma = np.where(vert, -x1, c).astype(np.float64)
    flip = alpha < 0
    alpha = np.where(flip, -alpha, alpha)
    beta = np.where(flip, -beta, beta)
    gamma = np.where(flip, -gamma, gamma)
    alpha = np.where(alpha == 0.0, 1e-12, alpha)
    u = (-beta / alpha).astype(np.float32)
    v = (-gamma / alpha).astype(np.float32)
    return u, v


def _finish(inter, aP, aG):
    inter = inter.astype(np.float64)
    aP = aP.astype(np.float64)
    aG = aG.astype(np.float64)
    vp = (aP > 0).astype(np.float64)
    vg = (aG > 0).astype(np.float64)
    uni = aP[:, None] + aG[None, :] - inter
    iou = np.where(uni > 0, inter / np.where(uni > 0, uni, 1.0), 0.0)
    row = np.sum(np.max(iou, axis=1) * vp)
    col = np.sum(np.max(iou, axis=0) * vg)
    return np.float32((row + col) / (vp.sum() + vg.sum()))


def _make_uvy(pred_pts, gt_pts):
    u, v = _host_params(pred_pts, gt_pts)
    maps = []
    for c in range(_NCORES):
        m = np.zeros((128, 33), np.float32)
        m[:, 0:16] = u[None, :]
        m[:, 16:32] = v[None, :]
        m[:, 32] = np.minimum(np.arange(128), _ROWS - 1) + _ROWS * c
        maps.append(m)
    return maps


def kernel(pred_pts, gt_pts):
    from concourse.bass_utils import run_bass_kernel_spmd

    if "nc" not in _CACHE:
        _CACHE["nc"] = _build_nc()
    nc = _CACHE["nc"]

    in_maps = [{"uvy": m} for m in _make_uvy(pred_pts, gt_pts)]
    res = run_bass_kernel_spmd(nc, in_maps, core_ids=list(range(_NCORES)))
    inter = np.zeros((256, 256), np.float64)
    aP = np.zeros(256, np.float64)
    aG = np.zeros(256, np.float64)
    for r in res.results:
        inter += r["out_inter"]
        aP += r["out_ap"].T.reshape(256)
        aG += r["out_ag"][0]
    return _finish(inter, aP, aG)
